# revision 19
# baseline (speedup 1.0000x reference)
"""Multi-head self-attention (B=1, S=4096, D=2048, H=16, rotary_dim=64) on 8 TRN2 NeuronCores.

Head-sharded tensor parallelism: each core computes 2 heads end-to-end
(QKV projection + RoPE + full softmax attention) plus its slice of the
row-sharded output projection; the 8 partial [S, D] outputs (fp16) are
summed on the host.

Schedule (v5): slot-based attention iterations with deferred tails.
  A: QKV+RoPE head 0
  B: attention(head0, qt) with fillers = tail(qt-1) + QKV+RoPE(head1, qt)
  C: attention(head1, qt) with fillers = tail(qt-1)  (pure attention)
  D: output projection for all q-tiles (pure GEMM + evictions)
Per-iteration softmax tail (denominator fold / ones-matmul / reciprocal /
normalize) is emitted as the first fillers of the NEXT iteration so the
PE queue never blocks on the reduction chain.

Design points:
  - denominator = one ones-matmul over accAll (all 32 exp chunks folded
    by DVE/gpsimd adds) -> broadcast across partitions in psum.
  - out-proj runs as its own stage D: interleaving it with attention
    (v2-v4) congested the scalar/vector engines whose psum evictions
    pace the score pipeline, stretching every matmul ~8%.
  - x is host-pre-tiled to [st][p][k][s] so every DMA line is >=2KB
    contiguous (16KB for the half-tile loads).
  - startup loads only head 0's weight columns; head 1's half arrives
    during stiles 3-6. All startup x/w DMA on the two HWDGE queues
    (sync/scalar); gpsimd DMA issue costs ~1us on-engine.
"""

import numpy as np

import concourse.bass as bass
import concourse.mybir as mybir
import concourse.tile as tile
from concourse import bacc
from concourse.bass_utils import run_bass_kernel_spmd
from concourse.masks import make_identity

F32 = mybir.dt.float32
FP16 = mybir.dt.float16

D = 2048
H = 16
HD = 128
ROT = 64
NCORES = 8
HPC = H // NCORES  # heads per core
SCALE = float(HD) ** -0.5

_CACHE = {}


def build_module(S=4096, ST=512, QTL=512):
    NST = S // ST        # QKV s-tiles
    NKT = D // 128       # contraction tiles for QKV
    NQT = S // QTL       # attention q-tiles
    NKC = S // 128       # attention k-chunks
    NPAIR = NKC // 2     # score-pair slots per attention iteration
    ETL = 512            # out-proj e-tile
    NET = D // ETL
    PLAG = 4             # lag (in pairs) between scores and AV consumption
    NSLOT = NPAIR + PLAG
    assert NST == NQT

    nc = bacc.Bacc(None, target_bir_lowering=False, debug=True)

    xT_d = nc.dram_tensor("xT", [NST, 128, NKT, ST], FP16, kind="ExternalInput")
    w_d = nc.dram_tensor("wsl", [D, 3 * HPC, 128], FP16, kind="ExternalInput")
    wo_d = nc.dram_tensor("wout", [HPC * HD, D], FP16, kind="ExternalInput")
    b_d = nc.dram_tensor("bsl", [128, 3 * HPC], F32, kind="ExternalInput")
    cs_d = nc.dram_tensor("cs", [ROT, 2, S], FP16, kind="ExternalInput")
    y_d = nc.dram_tensor("y", [S, D], FP16, kind="ExternalOutput")

    w_r = w_d[:].rearrange("(t p) j m -> p t j m", p=128)
    wo_r = wo_d[:].rearrange("(t p) e -> p t e", p=128)

    with tile.TileContext(nc) as tc:
        with (
            tc.tile_pool(name="persist", bufs=1) as P,
            tc.tile_pool(name="wp", bufs=1) as wp,
            tc.tile_pool(name="xp", bufs=2) as xp,
            tc.tile_pool(name="csp", bufs=2) as csp,
            tc.tile_pool(name="vtp", bufs=2) as vtp,
            tc.tile_pool(name="rtp", bufs=2) as rtp,
            tc.tile_pool(name="ptp", bufs=5) as ptp,
            tc.tile_pool(name="accDp", bufs=2) as accDp,
            tc.tile_pool(name="accGp", bufs=2) as accGp,
            tc.tile_pool(name="accfp", bufs=2) as accfp,
            tc.tile_pool(name="rcp", bufs=2) as rcp,
            tc.tile_pool(name="yp", bufs=6) as yp,
            tc.tile_pool(name="pss", bufs=3, space="PSUM") as pssp,
            tc.tile_pool(name="pop", bufs=2, space="PSUM") as pop,
            tc.tile_pool(name="ps1", bufs=2, space="PSUM") as ps1,
        ):
            QT = [P.tile([128, S], FP16, tag=f"qt{h}", name=f"qt{h}") for h in range(HPC)]
            KT = [P.tile([128, S], FP16, tag=f"kt{h}", name=f"kt{h}") for h in range(HPC)]
            V = [P.tile([128, NKC, 128], FP16, tag=f"v{h}", name=f"v{h}") for h in range(HPC)]
            AT = [P.tile([128, S], FP16, tag=f"at{h}", name=f"at{h}") for h in range(HPC)]
            b_sb = P.tile([128, 3 * HPC], F32)
            identr = P.tile([128, 128], FP16)
            ones_r = P.tile([128, 128], FP16)
            wo_sb = P.tile([128, HPC, D], FP16)
            w_sb = wp.tile([128, NKT, 3 * HPC, 128], FP16)

            # memset first so the PE warmup can start immediately; all other
            # startup work (DMAs) is issued from t_load of the first s-tile.
            nc.vector.memset(ones_r[:], 1.0)

            # Warm the PE clock (HAM) during the initial weight/x DMAs so the
            # first real matmuls run at full frequency.
            NWARM = 48
            wm = pop.tile([128, 128], F32, tag="po", name="warm")
            for i in range(NWARM):
                nc.tensor.matmul(wm[:], ones_r[:], ones_r[:],
                                 start=(i == 0), stop=(i == NWARM - 1))

            nc.gpsimd.dma_start(b_sb[:], b_d[:])

            def qkv_stile_thunks(h, st):
                """QKV projection + RoPE + V transpose for one head / s-tile,
                returned as a list of emission thunks for interleaving."""
                sl = bass.ts(st, ST)
                j = 3 * h
                state = {}
                thunks = []

                def t_load():
                    xt = xp.tile([128, NKT, ST], FP16, tag="xt", name=f"xt_{h}_{st}")
                    cst = csp.tile([ROT, 2, ST], FP16, tag="cst", name=f"cst_{h}_{st}")
                    if h == 0 and st == 0:
                        # startup: only head 0's weight columns (j 0:3) now —
                        # head 1's half is deferred to st=3..6 — w/x k-pairs
                        # alternate between the two HWDGE queues in
                        # consumption order.
                        nc.gpsimd.dma_start(cst[:], cs_d[:, :, sl])
                        for e in range(8):
                            k0, k1 = 2 * e, 2 * e + 2
                            weng = nc.sync if e % 2 == 0 else nc.scalar
                            xeng = nc.scalar if e % 2 == 0 else nc.sync
                            weng.dma_start(w_sb[:, k0:k1, 0:3, :], w_r[:, k0:k1, 0:3, :])
                            xeng.dma_start(xt[:, k0:k1, :], xT_d[st, :, k0:k1, :])
                        make_identity(nc, identr)
                        # consume the warmup accumulator; emitted after the
                        # DMA issues so no issue queue waits on the warmup.
                        wmr = rcp.tile([128, 1], F32, tag="rc", name="warmread")
                        nc.scalar.activation(wmr[:], wm[:, 0:1],
                                             mybir.ActivationFunctionType.Copy)
                    else:
                        nc.sync.dma_start(xt[:, 0 : NKT // 2, :], xT_d[st, :, 0 : NKT // 2, :])
                        nc.scalar.dma_start(xt[:, NKT // 2 :, :], xT_d[st, :, NKT // 2 :, :])
                        nc.sync.dma_start(cst[:], cs_d[:, :, sl])
                        if h == 0 and 3 <= st <= 6:
                            # deferred head-1 QKV weight columns (not needed
                            # until stage B), one k-quarter per s-tile
                            ksl = slice(4 * (st - 3), 4 * (st - 2))
                            eng = nc.sync if st % 2 else nc.scalar
                            eng.dma_start(w_sb[:, ksl, 3:6, :], w_r[:, ksl, 3:6, :])
                    state["xt"] = xt
                    state["cst"] = cst
                thunks.append(t_load)

                def t_group_open(which):
                    # startup opens q/k/v simultaneously; v borrows the dn
                    # psum bank (unused until stage B) so ps1 stays at 2 bufs.
                    tg = "dnps" if (h == 0 and st == 0 and which == 2) else "ps1"
                    bf = 1 if tg == "dnps" else None
                    state[f"ps{which}"] = ps1.tile(
                        [128, ST], F32, tag=tg, bufs=bf, name=f"ps{which}_{h}_{st}")
                def t_mms(which, k0, k1):
                    ps = state[f"ps{which}"]
                    xt = state["xt"]
                    for k in range(k0, k1):
                        nc.tensor.matmul(
                            ps[:], w_sb[:, k, j + which, :], xt[:, k, :],
                            start=(k == 0), stop=(k == NKT - 1),
                        )
                def t_evict_qk(which):
                    dst = QT[h] if which == 0 else KT[h]
                    cst = state["cst"]
                    nc.scalar.activation(
                        dst[:, sl], state[f"ps{which}"][:],
                        mybir.ActivationFunctionType.Identity,
                        bias=b_sb[:, j + which : j + which + 1],
                    )
                    tmp = rtp.tile([ROT, ST], FP16, tag="rtmp", name=f"rt_{h}_{st}_{which}")
                    nc.vector.tensor_copy(tmp[0 : ROT // 2, :], dst[ROT // 2 : ROT, sl])
                    nc.vector.tensor_copy(tmp[ROT // 2 : ROT, :], dst[0 : ROT // 2, sl])
                    nc.vector.tensor_mul(tmp[:], tmp[:], cst[:, 1, :])
                    nc.vector.tensor_mul(dst[0:ROT, sl], dst[0:ROT, sl], cst[:, 0, :])
                    nc.vector.tensor_add(dst[0:ROT, sl], dst[0:ROT, sl], tmp[:])
                def t_evict_v():
                    vt = vtp.tile([128, ST], FP16, tag="vt", name=f"vt_{h}_{st}")
                    nc.scalar.activation(
                        vt[:], state["ps2"][:], mybir.ActivationFunctionType.Identity,
                        bias=b_sb[:, j + 2 : j + 3],
                    )
                    state["vt"] = vt
                def t_vtr():
                    ptr4 = ps1.tile([128, ST // 128, 128], FP16, tag="ps1",
                                    name=f"ptr_{h}_{st}")
                    for sc in range(ST // 128):
                        nc.tensor.transpose(ptr4[:, sc, :],
                                            state["vt"][:, bass.ts(sc, 128)], identr[:])
                    state["ptr4"] = ptr4
                def t_vtr_evict():
                    nc.scalar.activation(
                        V[h][:, st * (ST // 128) : (st + 1) * (ST // 128), :],
                        state["ptr4"][:], mybir.ActivationFunctionType.Copy)

                if h == 0 and st == 0:
                    # startup: all three groups advance one k-pair at a time so
                    # the PE fully consumes each eighth-sized DMA arrival.
                    for which in range(3):
                        thunks.append(lambda w=which: t_group_open(w))
                    for k0 in range(0, NKT, 2):
                        for which in range(3):
                            thunks.append(lambda w=which, a=k0: t_mms(w, a, a + 2))
                    thunks.append(lambda: t_evict_qk(0))
                    thunks.append(lambda: t_evict_qk(1))
                    thunks.append(t_evict_v)
                else:
                    for which in range(2):
                        thunks.append(lambda w=which: t_group_open(w))
                        for k0 in range(0, NKT, 4):
                            thunks.append(lambda w=which, a=k0: t_mms(w, a, a + 4))
                        thunks.append(lambda w=which: t_evict_qk(w))
                    thunks.append(lambda: t_group_open(2))
                    for k0 in range(0, NKT, 4):
                        thunks.append(lambda a=k0: t_mms(2, a, a + 4))
                    thunks.append(t_evict_v)
                thunks.append(t_vtr)
                thunks.append(t_vtr_evict)
                return thunks

            def outproj_group(qt, sc4):
                """Out-projection for one 128-row chunk: 4 psy accumulations
                into one [128, D] staging tile, a single 4KB-per-partition
                DMA out (128 separate DMA issues would outrun the sync DGE)."""
                ssl = bass.ds(qt * QTL + sc4 * 128, 128)
                ytg = yp.tile([128, NET, ETL], FP16, tag="yt", bufs=3,
                              name=f"yt_{qt}_{sc4}")
                for et in range(NET):
                    esl = bass.ts(et, ETL)
                    psy = pssp.tile([128, ETL], F32, tag="pss",
                                    name=f"psy_{qt}_{sc4}_{et}")
                    for h in range(HPC):
                        nc.tensor.matmul(
                            psy[:], AT[h][:, ssl], wo_sb[:, h, esl],
                            start=(h == 0), stop=(h == HPC - 1),
                        )
                    # split evictions across vector/scalar so neither engine
                    # falls behind the psy matmul stream
                    if et % 2 == 0:
                        nc.vector.tensor_copy(ytg[:, et, :], psy[:])
                    else:
                        nc.scalar.activation(ytg[:, et, :], psy[:],
                                             mybir.ActivationFunctionType.Copy)
                nc.sync.dma_start(y_d[ssl, :], ytg[:])

            def attn_slots(h, qt, fillers, startf=0):
                """One attention iteration (512 queries x full S keys) emitted
                as NSLOT slots with `fillers` paced across slots [startf, NSLOT).
                Returns tail thunks (denominator fold + ones-matmul +
                normalize) to be emitted as fillers of the NEXT iteration."""
                fillers = list(fillers)
                fi = 0
                total = NSLOT - startf
                qsl = bass.ts(qt, QTL)
                oacc = pop.tile([128, QTL], F32, tag="po", name=f"oacc_{h}_{qt}")
                accD = accDp.tile([128, 4, QTL], FP16, tag="accD", name=f"aD_{h}_{qt}")
                accG = accGp.tile([128, 4, QTL], FP16, tag="accG", name=f"aG_{h}_{qt}")
                quads = {}
                accPre = accfp.tile([128, 2, QTL], FP16, tag="accPre",
                                    name=f"ap_{h}_{qt}")
                accf2 = accfp.tile([128, QTL], FP16, tag="accf2",
                                   name=f"af2_{h}_{qt}")
                accP7 = accfp.tile([128, 2, QTL], FP16, tag="accP7",
                                   name=f"ap7_{h}_{qt}")
                accAll = accfp.tile([128, QTL], FP16, tag="accAll",
                                    name=f"aall_{h}_{qt}")
                dnh = {}
                for sl in range(NSLOT):
                    if sl < NPAIR:
                        p = sl
                        q = p // 2
                        if p % 2 == 0:
                            quads[q] = ptp.tile([128, 4, QTL], FP16, tag="pt",
                                                name=f"pt_{h}_{qt}_{q}")
                        for half in range(2):
                            kc = 2 * p + half
                            pss = pssp.tile([128, QTL], F32, tag="pss",
                                            name=f"pss_{h}_{qt}_{kc}")
                            nc.tensor.matmul(
                                pss[:], KT[h][:, bass.ts(kc, 128)],
                                QT[h][:, qsl], start=True, stop=True,
                            )
                            nc.scalar.activation(
                                quads[q][:, 2 * (p % 2) + half, :], pss[:],
                                mybir.ActivationFunctionType.Exp, scale=SCALE,
                            )
                    # denominator partial reduction (DVE + GpSimd); quad 7 is
                    # folded in the tail so the last exp -> reduce chain is
                    # short.
                    if sl == 4:
                        nc.vector.tensor_add(accD[:], quads[0][:], quads[1][:])
                    elif sl == 8:
                        nc.gpsimd.tensor_add(accG[:], quads[2][:], quads[3][:])
                    elif sl == 10:
                        nc.vector.tensor_add(accD[:], accD[:], quads[4][:])
                    elif sl == 12:
                        nc.vector.tensor_add(accD[:], accD[:], quads[5][:])
                    elif sl == 14:
                        nc.vector.tensor_add(accD[:], accD[:], quads[6][:])
                    elif sl == 15:
                        nc.vector.tensor_add(accD[:], accD[:], accG[:])
                    elif sl == 16:
                        nc.vector.tensor_add(accPre[:], accD[:, 0:2, :], accD[:, 2:4, :])
                    elif sl == 17:
                        nc.vector.tensor_add(accf2[:], accPre[:, 0, :], accPre[:, 1, :])
                    if sl >= startf:
                        while fi < len(fillers) and fi * total <= (sl - startf + 1) * len(fillers):
                            fillers[fi]()
                            fi += 1
                    pd = sl - PLAG
                    if 0 <= pd < NPAIR:
                        for half in range(2):
                            kd = 2 * pd + half
                            nc.tensor.matmul(
                                oacc[:], V[h][:, kd, :], quads[kd // 4][:, kd % 4, :],
                                start=(kd == 0), stop=(kd == NKC - 1),
                            )
                while fi < len(fillers):
                    fillers[fi]()
                    fi += 1
                q7 = quads[7]

                def t_tail():
                    # fold quad 7 into the running sum, then reduce across
                    # partitions with a single ones-matmul (dn broadcast to
                    # all 128 partitions in psum).
                    nc.vector.tensor_add(accP7[:], q7[:, 0:2, :], q7[:, 2:4, :])
                    nc.vector.tensor_add(accAll[:], accP7[:, 0, :], accP7[:, 1, :])
                    nc.vector.tensor_add(accAll[:], accAll[:], accf2[:])
                    dn = ps1.tile([128, QTL], F32, tag="dnps", bufs=1,
                                  name=f"dn_{h}_{qt}")
                    nc.tensor.matmul(dn[:], ones_r[:], accAll[:],
                                     start=True, stop=True)
                    dnh["dn"] = dn
                def t_norm():
                    rc = rcp.tile([128, QTL], F32, tag="rc", name=f"rc_{h}_{qt}")
                    scr = rcp.tile([128, QTL], F32, tag="rcscr", name=f"rs_{h}_{qt}")
                    nc.vector.reciprocal_approx_accurate(rc[:], dnh["dn"][:], scr[:])
                    nc.vector.tensor_mul(AT[h][:, qsl], oacc[:], rc[:])
                return [t_tail, t_norm]

            def t_wo():
                nc.gpsimd.dma_start(wo_sb[:], wo_r)

            # stage A: QKV head 0
            for st in range(NST):
                for t in qkv_stile_thunks(0, st):
                    t()
            # stage B: attention(head0) || QKV head 1, tails deferred
            tail = []
            for qt in range(NQT):
                fillers = tail + qkv_stile_thunks(1, qt)
                if qt == 5:
                    fillers = fillers + [t_wo]
                tail = attn_slots(0, qt, fillers)
            # stage C: attention(head1), pure
            for qt in range(NQT):
                tail = attn_slots(1, qt, list(tail), startf=2)
            # stage D: output projection for all q-tiles
            for t in tail:
                t()
            for qt in range(NQT):
                for sc4 in range(QTL // 128):
                    outproj_group(qt, sc4)

    nc.compile()
    return nc


def _host_prep(x, w_qkv, b_qkv, w_out, S):
    """Build per-core input maps."""
    NST, NKT, ST = S // 512, D // 128, 512
    xT = np.ascontiguousarray(x.reshape(S, D).T).astype(np.float16)   # [D, S]
    # pre-tile to [st][p][k][s] so each partition's DMA line is contiguous
    xtl = np.ascontiguousarray(
        xT.reshape(NKT, 128, NST, ST).transpose(2, 1, 0, 3))          # [st,p,k,s]

    # RoPE tables (match reference._rope_cos_sin)
    inv_freq = (1.0 / (10000.0 ** (np.arange(0, ROT, 2, dtype=np.float32) / ROT))).astype(np.float32)
    t = np.arange(S, dtype=np.float32)
    freqs = np.outer(t, inv_freq)                      # [S, ROT/2]
    emb = np.concatenate([freqs, freqs], axis=-1)      # [S, ROT]
    cosT = np.cos(emb).astype(np.float32).T            # [ROT, S]
    sinT = np.sin(emb).astype(np.float32).T
    sinS = sinT.copy()
    sinS[0 : ROT // 2] *= -1.0
    cs = np.ascontiguousarray(np.stack([cosT, sinS], axis=1)).astype(np.float16)  # [ROT, 2, S]

    in_maps = []
    for c in range(NCORES):
        cols = []
        bcols = []
        for h in [HPC * c + i for i in range(HPC)]:
            for part in range(3):  # q, k, v
                off = part * D + h * HD
                cols.append(w_qkv[:, off : off + HD])
                bcols.append(b_qkv[off : off + HD])
        wsl = np.ascontiguousarray(np.stack(cols, axis=1)).astype(np.float16)   # [D, 3*HPC, 128]
        bsl = np.ascontiguousarray(np.stack(bcols, axis=1)).astype(np.float32)  # [128, 3*HPC]
        wout_sl = np.ascontiguousarray(w_out[c * HPC * HD : (c + 1) * HPC * HD, :]).astype(np.float16)
        in_maps.append({"xT": xtl, "wsl": wsl, "bsl": bsl, "wout": wout_sl, "cs": cs})
    return in_maps


def kernel(x, w_qkv, b_qkv, w_out, b_out):
    B, S, D_ = x.shape
    assert B == 1 and D_ == D
    if "nc" not in _CACHE:
        _CACHE["nc"] = build_module(S=S)
    nc = _CACHE["nc"]
    in_maps = _host_prep(np.asarray(x, dtype=np.float32), np.asarray(w_qkv, dtype=np.float32),
                         np.asarray(b_qkv, dtype=np.float32), np.asarray(w_out, dtype=np.float32), S)
    res = run_bass_kernel_spmd(nc, in_maps, list(range(NCORES)))
    y = np.zeros((S, D), dtype=np.float32)
    for c in range(NCORES):
        y += res.results[c]["y"].astype(np.float32)
    y += np.asarray(b_out, dtype=np.float32)[None, :]
    return y.reshape(1, S, D)
